# revision 9
# baseline (speedup 1.0000x reference)
"""Trainium2 Bass kernel for nn_Llama3 (8 layers, B=2, S=1024, D=768).

Sharding: DP=2 over batch x CP=4 over sequence (256 tokens/core).
  - activations live feature-major [128, D/128, T] per core
  - per layer: local K/V proj + rope -> AllGather (K feature-major,
    V token-major) within each CP group of 4, overlapped with the local
    Q projection and the diagonal attention tiles; full-width causal
    attention via additive masks / exp biases; local FFN.
  - LM head: LOCAL tokens x FULL vocab per core (32256 padded cols
    streamed in 252 weight tiles) -> per-token sum(exp) and label logits
    computed entirely locally; no end-of-model collectives.  Host
    combines the 8 per-token NLL vectors.
Attention packs the 4 query heads sharing each KV head into single
N=1024 matmuls.

FP8 path: all weight-stationary matmuls (q/k/v/o, FFN, LM head) run in
fp8e4 with MatmulPerfMode.DoubleRow (contraction 256/instruction, ~2x
bf16 throughput).  Activations are pre-normalized (rmsnorm applied
BEFORE the projection) and cast to fp8 with scale AS=16; weights carry
scale WS=32 (folded on host); the 1/(AS*WS) descale is folded into the
existing post-matmul vector op.  Attention scores/AV stay bf16.
"""

import sys

sys.path.insert(0, "/opt/trn_rl_repo")

import numpy as np
import ml_dtypes

import concourse.bass as bass
import concourse.mybir as mybir
import concourse.tile as tile
from concourse import bacc
from concourse import bass_utils
from concourse.masks import make_identity

# ---- model constants (hardcoded per problem spec) ----
P = 128
B, S, D, H, G, L, V = 2, 1024, 768, 12, 4, 8, 32000
HD = D // H            # 64
KV = H // G            # 3 kv heads
KVD = KV * HD          # 192
HID = 2048
EOS = 2
EPS = 1.1920929e-07    # float32 eps (torch RMSNorm eps=None)
NEG = -30000.0

R = 4                  # CP degree (sequence chunks)
NC = 8                 # cores
T = S // R             # 256 local tokens
T4 = 4 * T             # packed attention free dim (4 heads)
DT = D // P            # 6 feature tiles
HT = HID // P          # 16
VPAD = 32256           # padded vocab (252 * 128)
VT = VPAD // P         # 252 vocab tiles

bf16 = mybir.dt.bfloat16
f8 = mybir.dt.float8e4
f32 = mybir.dt.float32
BF = ml_dtypes.bfloat16
F8 = ml_dtypes.float8_e4m3
AF = mybir.ActivationFunctionType
OP = mybir.AluOpType
DR = mybir.MatmulPerfMode.DoubleRow

AS = 16.0              # activation fp8 scale (folded into rsqrt bias)
WS = 32.0              # weight fp8 scale (folded on host)
FS = 4.0               # ffa (silu*v) fp8 scale
D1 = 1.0 / (AS * WS)   # post-projection descale
LN_AS = float(np.log(AS))

REPLICA_GROUPS = [[0, 1, 2, 3], [4, 5, 6, 7]]

# AllGather payload layout (per rank, bf16 elements):
K_SZ = 64 * KV * 256   # k64 [64, 3, 256]
V_SZ = P * 2 * 195     # v token-major [128, 2, 3*65]
AG_SZ = K_SZ + V_SZ    # 99072


def build_program(num_layers=L, single_core=False, mock_collectives=False,
                  reps=1, hw_loop=None):
    nc = bacc.Bacc("TRN2", target_bir_lowering=False, debug=False,
                   enable_asserts=False, num_devices=1 if single_core else NC)

    def collective(kind, op, ins, outs):
        if not single_core and not mock_collectives:
            nc.gpsimd.collective_compute(kind, op, replica_groups=REPLICA_GROUPS,
                                         ins=ins, outs=outs)
            return
        in_ap, out_ap = ins[0], outs[0]
        n = in_ap.size()
        nblk = out_ap.size() // n
        for b_ in range(nblk):
            nc.sync.dma_start(out_ap.tensor.ap()[b_ * n:(b_ + 1) * n], in_ap)

    # ---------------- DRAM I/O ----------------
    def din(name, shape, dt):
        return nc.dram_tensor(name, list(shape), dt, kind="ExternalInput").ap()

    x0_d = din("x0", (P, DT, T), f32)
    wq_d = din("wq", (num_layers, P, DT, D), f8)
    wk_d = din("wk", (num_layers, P, DT, KVD), f8)
    wv_d = din("wv", (num_layers, P, DT, KVD), f8)
    wo_d = din("wo", (num_layers, P, DT, D), f8)
    w1_d = din("w1", (num_layers, P, DT, HID), f8)
    vw_d = din("vw", (num_layers, P, DT, HID), f8)
    w2_d = din("w2", (num_layers, P, HT, D), f8)
    bq_d = din("bq", (num_layers, P, DT), f32)
    bk_d = din("bk", (num_layers, P, 2), f32)
    bv_d = din("bv", (num_layers, P, 2), f32)
    bo_d = din("bo", (num_layers, P, DT), f32)
    c1q_d = din("c1q", (P, T), bf16)
    c2q_d = din("c2q", (P, T), bf16)
    c1k_d = din("c1k", (P, T), bf16)
    c2k_d = din("c2k", (P, T), bf16)
    maskc_d = din("maskc", (P, 2, 2 * T), f32)
    biasj_d = din("biasj", (P, 2 + 2 * R), f32)
    sel_d = din("sel", (12, D), bf16)
    lmw_d = din("lmw", (VT, P, DT, P), f8)
    lmbv_d = din("lmbv", (P, VT), f32)
    wlab_d = din("wlab", (P, DT, T), bf16)
    lmblab_d = din("lmblab", (1, T), f32)

    nll_d = nc.dram_tensor("nll", [1, T], f32, kind="ExternalOutput").ap()

    from contextlib import ExitStack
    with tile.TileContext(nc) as tc, ExitStack() as ctx:
        pconst = ctx.enter_context(tc.tile_pool(name="pconst", bufs=1))
        pstate = ctx.enter_context(tc.tile_pool(name="pstate", bufs=1))
        pw = ctx.enter_context(tc.tile_pool(name="pw", bufs=2))
        pact = ctx.enter_context(tc.tile_pool(name="pact", bufs=1))
        ptmp = ctx.enter_context(tc.tile_pool(name="ptmp", bufs=4))
        pexp = ctx.enter_context(tc.tile_pool(name="pexp", bufs=6))
        pdram = ctx.enter_context(tc.tile_pool(name="pdram", bufs=1, space="DRAM"))
        # PSUM: 16KB/partition total.  mmt: 6 x 2KB slots; pso: 2 x 2KB.
        pp_mm = ctx.enter_context(tc.tile_pool(name="ppmm", bufs=6, space="PSUM"))
        pp_o = ctx.enter_context(tc.tile_pool(name="ppo", bufs=2, space="PSUM"))

        # ---- constants (loaded once; shared by all reps) ----
        ones_bf = pconst.tile([P, 1], bf16, name="ones_bf")
        nc.vector.memset(ones_bf[:], 1.0)
        ones1_bf = pconst.tile([1, P], bf16, name="ones1_bf")
        nc.vector.memset(ones1_bf[:], 1.0)
        ident_bf = pconst.tile([P, P], bf16, name="ident_bf")
        make_identity(nc, ident_bf[:])
        eps_col = pconst.tile([P, 1], f32, name="eps_col")
        nc.vector.memset(eps_col[:], EPS)
        lnas_col = pconst.tile([P, 1], f32, name="lnas_col")
        nc.vector.memset(lnas_col[:], LN_AS)

        def load_const(name, ap, shape, dt):
            t = pconst.tile(list(shape), dt, name=name)
            nc.sync.dma_start(t[:], ap)
            return t

        c1q = load_const("c1q_s", c1q_d[:], (P, T), bf16)
        c2q = load_const("c2q_s", c2q_d[:], (P, T), bf16)
        c1k = load_const("c1k_s", c1k_d[:], (P, T), bf16)
        c2k = load_const("c2k_s", c2k_d[:], (P, T), bf16)
        maskc_sb = load_const("maskc_s", maskc_d[:], (P, 2, 2 * T), f32)
        biasj_sb = load_const("biasj_s", biasj_d[:], (P, 2 + 2 * R), f32)
        sel_sb = load_const("sel_s", sel_d[:], (12, D), bf16)
        wlab_sb = load_const("wlab_s", wlab_d[:], (P, DT, T), bf16)
        lmblab_sb = load_const("lmblab_s", lmblab_d[:], (1, T), f32)
        lmbv_sb = load_const("lmbv_s", lmbv_d[:], (P, VT), f32)

        # ---------------- helpers ----------------
        def norm_cast(x, out8, tag):
            """x [P, DT, T] f32 -> out8 [P, DT, T] fp8 = AS * x * rsqrt(
            mean(x^2) + eps).  Norm weights are folded into the following
            projection weights on host.  rsqrt via ln+exp so the Act engine
            stays on the {exp,ln} table; AS folds into the exp bias."""
            ps_ss = pp_o.tile([1, T], f32, name=f"ss_{tag}", tag="pso")
            for i in range(DT):
                xsq = ptmp.tile([P, T], bf16, name=f"xsq_{tag}_{i}", tag="xsq")
                nc.vector.tensor_tensor(xsq[:], x[:, i, :], x[:, i, :], OP.mult)
                nc.tensor.matmul(ps_ss[:], ones_bf[:], xsq[:],
                                 start=(i == 0), stop=(i == DT - 1))
            ln_ms = ptmp.tile([1, T], f32, name=f"ln_{tag}", tag="rowtmp")
            nc.scalar.activation(ln_ms[:], ps_ss[:], AF.Ln, bias=eps_col[0:1, :],
                                 scale=1.0 / D)
            rs = ptmp.tile([1, T], bf16, name=f"rs_{tag}", tag="rowtmp")
            with nc.allow_low_precision(reason="bf16 rsqrt scale for bcast matmul"):
                nc.scalar.activation(rs[:], ln_ms[:], AF.Exp, scale=-0.5,
                                     bias=lnas_col[0:1, :])
            ps_bc = pp_mm.tile([P, T], f32, name=f"bc_{tag}", tag="mmt")
            nc.tensor.matmul(ps_bc[:], ones1_bf[:], rs[:], start=True, stop=True)
            rs_sb = ptmp.tile([P, T], f32, name=f"rsbc_{tag}", tag="rsbc")
            nc.vector.tensor_copy(rs_sb[:], ps_bc[:])
            for i in range(DT):
                nc.vector.tensor_tensor(out8[:, i, :], x[:, i, :], rs_sb[:],
                                        OP.mult)

        def rope_to64(raw, c1, c2, outs, tag):
            """raw [128,T] bf16 (2 heads, deinterleaved e/o per 64-block) ->
            rope'd halves written to the two [64,T] APs in `outs`."""
            sw = ptmp.tile([P, T], bf16, name=f"sw_{tag}", tag="ropesw")
            for blk in range(4):
                pr = (blk ^ 1) * 32
                nc.vector.tensor_copy(sw[blk * 32:(blk + 1) * 32, :],
                                      raw[pr:pr + 32, :])
            t1 = ptmp.tile([P, T], bf16, name=f"t1_{tag}", tag="ropet1")
            nc.vector.tensor_tensor(t1[:], raw[:], c1[:], OP.mult)
            t2 = ptmp.tile([P, T], bf16, name=f"t2_{tag}", tag="ropet2")
            nc.vector.tensor_tensor(t2[:], sw[:], c2[:], OP.mult)
            nc.vector.tensor_tensor(outs[0], t1[0:64, :], t2[0:64, :], OP.add)
            nc.vector.tensor_tensor(outs[1], t1[64:P, :], t2[64:P, :], OP.add)

        def rope64(raw, c1, c2, out, tag):
            """raw [*,T] bf16, rows 0..63 used (1 head) -> out [64,T] bf16"""
            sw = ptmp.tile([P, T], bf16, name=f"sw1_{tag}", tag="ropesw")
            nc.vector.tensor_copy(sw[0:32, :], raw[32:64, :])
            nc.vector.tensor_copy(sw[32:64, :], raw[0:32, :])
            t1 = ptmp.tile([P, T], bf16, name=f"t1a_{tag}", tag="ropet1")
            nc.vector.tensor_tensor(t1[0:64, :], raw[0:64, :], c1[0:64, :], OP.mult)
            t2 = ptmp.tile([P, T], bf16, name=f"t2a_{tag}", tag="ropet2")
            nc.vector.tensor_tensor(t2[0:64, :], sw[0:64, :], c2[0:64, :], OP.mult)
            nc.vector.tensor_tensor(out, t1[0:64, :], t2[0:64, :], OP.add)

        def mm8(ps_ap, w_sb, col0, cols, rhs8, npair=DT // 2):
            """accumulating DoubleRow chain: ps += w[:, :, col0:col0+cols].T
            @ rhs over `npair` feature-tile pairs."""
            for kk in range(npair):
                nc.tensor.matmul(ps_ap, w_sb[:, 2 * kk:2 * kk + 2,
                                             col0:col0 + cols],
                                 rhs8[:, 2 * kk:2 * kk + 2, :],
                                 start=(kk == 0), stop=(kk == npair - 1),
                                 perf_mode=DR)

        def model_body(rep):
            x_sb = pstate.tile([P, DT, T], f32, name=f"x_sb_r{rep}", tag="xsb")
            nc.sync.dma_start(x_sb[:], x0_d[:])

            # ---------------- transformer layers ----------------
            for l in range(num_layers):
                # K/V weights first (K/V proj gates the AllGather)
                wk_sb = pw.tile([P, DT, KVD], f8, name=f"wk{l}", tag="wkv")
                nc.sync.dma_start(wk_sb[:], wk_d[l])
                wv_sb = pw.tile([P, DT, KVD], f8, name=f"wv{l}", tag="wkv")
                nc.sync.dma_start(wv_sb[:], wv_d[l])
                bk_sb = pw.tile([P, 2], f32, name=f"bk{l}", tag="bk")
                nc.sync.dma_start(bk_sb[:], bk_d[l])
                bv_sb = pw.tile([P, 2], f32, name=f"bv{l}", tag="bk")
                nc.sync.dma_start(bv_sb[:], bv_d[l])
                wq_sb = pw.tile([P, DT, D], f8, name=f"wq{l}", tag="wqo")
                nc.sync.dma_start(wq_sb[:], wq_d[l])
                bq_sb = pw.tile([P, DT], f32, name=f"bq{l}", tag="bq")
                nc.sync.dma_start(bq_sb[:], bq_d[l])

                # pre-normalized fp8 activations (norm1 folded into wq/wk/wv)
                h1 = pact.tile([P, DT, T], f8, name=f"h1_{l}", tag="h1")
                norm_cast(x_sb, h1, f"n1l{l}")

                # ---- k projection + rope -> k64 [64, 3, T] ----
                k64 = pact.tile([64, KV, T], bf16, name=f"k64_{l}", tag="k64")
                ps = pp_mm.tile([P, T], f32, name=f"kp{l}_0", tag="mmt")
                mm8(ps[:], wk_sb, 0, P, h1)
                kraw = ptmp.tile([P, T], bf16, name=f"kraw{l}_0", tag="qraw")
                nc.vector.tensor_scalar(kraw[:], ps[:], D1, bk_sb[:, 0:1],
                                        OP.mult, OP.add)
                rope_to64(kraw, c1k, c2k, (k64[:, 0, :], k64[:, 1, :]), f"k{l}_0")
                ps = pp_mm.tile([P, T], f32, name=f"kp{l}_1", tag="mmt")
                mm8(ps[0:64, :], wk_sb, P, 64, h1)
                kraw = ptmp.tile([P, T], bf16, name=f"kraw{l}_1", tag="qraw")
                nc.vector.tensor_scalar(kraw[0:64, :], ps[0:64, :], D1,
                                        bk_sb[0:64, 1:2], OP.mult, OP.add)
                rope64(kraw, c1k, c2k, k64[:, 2, :], f"k{l}_1")

                # ---- v projection -> token-major with ones column ----
                vtm = pact.tile([P, 2, 195], bf16, name=f"vtm{l}", tag="vtm")
                nc.vector.memset(vtm[:], 0.0)
                vfm = ptmp.tile([P, 2, T], bf16, name=f"vfm{l}", tag="vfm")
                for m, rows in ((0, P), (1, 64)):
                    ps = pp_mm.tile([P, T], f32, name=f"vp{l}_{m}", tag="mmt")
                    mm8(ps[:rows, :], wv_sb, m * P, rows, h1)
                    nc.vector.tensor_scalar(vfm[:rows, m, :], ps[:rows, :], D1,
                                            bv_sb[:rows, m:m + 1],
                                            OP.mult, OP.add)
                for tj in range(2):
                    pst = pp_mm.tile([P, P], bf16, name=f"vt{l}_{tj}", tag="mmt")
                    nc.tensor.transpose(pst[:], vfm[:, 0, tj * P:(tj + 1) * P], ident_bf[:])
                    nc.scalar.copy(vtm[:, tj, 0:64], pst[:, 0:64])
                    nc.scalar.copy(vtm[:, tj, 65:129], pst[:, 64:128])
                    pst2 = pp_mm.tile([P, 64], bf16, name=f"vt2{l}_{tj}", tag="mmt")
                    nc.tensor.transpose(pst2[:], vfm[0:64, 1, tj * P:(tj + 1) * P],
                                        ident_bf[0:64, 0:64])
                    nc.scalar.copy(vtm[:, tj, 130:194], pst2[:, 0:64])
                nc.vector.memset(vtm[:, :, 64:65], 1.0)
                nc.vector.memset(vtm[:, :, 129:130], 1.0)
                nc.vector.memset(vtm[:, :, 194:195], 1.0)

                # ---- AllGather K,V across CP group (overlapped with Q/diag) ----
                agin = pdram.tile([AG_SZ], bf16, name=f"agin{l}", tag=f"agin{l}")
                agout = pdram.tile([R * AG_SZ], bf16, name=f"agout{l}",
                                   tag=f"agout{l}")
                nc.sync.dma_start(
                    agin[0:K_SZ].rearrange("(p h t) -> p h t", p=64, h=KV), k64[:])
                nc.sync.dma_start(
                    agin[K_SZ:AG_SZ].rearrange("(p j e) -> p j e", p=P, j=2),
                    vtm[:])
                collective("AllGather", OP.bypass, [agin[:].opt()], [agout[:].opt()])
                kg = pact.tile([64, R, KV, T], bf16, name=f"kg{l}", tag="kg")
                vg = pact.tile([P, R, 2, 195], bf16, name=f"vg{l}", tag="vg")
                vg8 = pact.tile([P, R, 2, 208], f8, name=f"vg8{l}", tag="vg8")
                for b in range(R):
                    base = b * AG_SZ
                    nc.sync.dma_start(
                        kg[:, b, :, :],
                        agout[base:base + K_SZ].rearrange("(p h t) -> p h t", p=64, h=KV))
                    nc.sync.dma_start(
                        vg[:, b, :, :],
                        agout[base + K_SZ:base + AG_SZ].rearrange(
                            "(p j e) -> p j e", p=P, j=2))
                    nc.vector.tensor_copy(vg8[:, b, :, 0:195], vg[:, b, :, :])
                vtm8 = pact.tile([P, 2, 208], f8, name=f"vtm8{l}", tag="vtm8")
                nc.vector.tensor_copy(vtm8[:, :, 0:195], vtm[:])

                # ---- q projection + rope (overlaps the AllGather) ----
                q64 = pact.tile([64, H, T], bf16, name=f"q64_{l}", tag="q64")
                for m in range(DT):
                    ps = pp_mm.tile([P, T], f32, name=f"qp{l}_{m}", tag="mmt")
                    mm8(ps[:], wq_sb, m * P, P, h1)
                    qraw = ptmp.tile([P, T], bf16, name=f"qraw{l}_{m}", tag="qraw")
                    nc.vector.tensor_scalar(qraw[:], ps[:], D1, bq_sb[:, m:m + 1],
                                            OP.mult, OP.add)
                    rope_to64(qraw, c1q, c2q,
                              (q64[:, 2 * m, :], q64[:, 2 * m + 1, :]), f"q{l}_{m}")

                # prefetch next-phase weights while AG / attention run
                wo_sb = pw.tile([P, DT, D], f8, name=f"wo{l}", tag="wqo")
                nc.sync.dma_start(wo_sb[:], wo_d[l])
                bo_sb = pw.tile([P, DT], f32, name=f"bo{l}", tag="bq")
                nc.sync.dma_start(bo_sb[:], bo_d[l])
                w1h, vwh, w2h = [], [], []
                for hf in range(2):
                    w1_sb = pw.tile([P, DT, HID // 2], f8, name=f"w1{l}_{hf}",
                                    tag="wbig", bufs=3)
                    nc.sync.dma_start(
                        w1_sb[:], w1_d[l, :, :, hf * (HID // 2):(hf + 1) * (HID // 2)])
                    vw_sb = pw.tile([P, DT, HID // 2], f8, name=f"vw{l}_{hf}",
                                    tag="wbig", bufs=3)
                    nc.sync.dma_start(
                        vw_sb[:], vw_d[l, :, :, hf * (HID // 2):(hf + 1) * (HID // 2)])
                    w1h.append(w1_sb)
                    vwh.append(vw_sb)
                for hf in range(2):
                    w2_sb = pw.tile([P, HT // 2, D], f8, name=f"w2{l}_{hf}",
                                    tag="wbig", bufs=3)
                    nc.sync.dma_start(
                        w2_sb[:], w2_d[l, :, hf * (HT // 2):(hf + 1) * (HT // 2), :])
                    w2h.append(w2_sb)

                # ---- attention: head pairs (N = 2T); PSUM tiles stay within
                # one 2KB bank.  The exp->AV of key tile t is emitted after
                # the scores of tile t+1 so PE never waits on Act.
                # All diagonal scores/exps use only LOCAL k/v and are emitted
                # first, overlapping the in-flight AllGather. ----
                T2 = 2 * T
                o_sb = pact.tile([P, DT, T], f32, name=f"osb{l}", tag="osb")
                sums = pact.tile([12, T], f32, name=f"sums{l}", tag="sums")
                exd = {}
                for g in range(H // 2):
                    kvh = g // 2
                    q_rhs = q64[:, 2 * g:2 * g + 2, :]           # [64, 2, T]
                    ex2d = pexp.tile([P, 2, T2], f8, name=f"exd{l}_{g}",
                                     tag="expd", bufs=6)
                    for c in range(2):
                        ps_s = pp_mm.tile([P, T2], f32,
                                          name=f"psd{l}_{g}_{c}", tag="mmt")
                        nc.tensor.matmul(ps_s[:], k64[:, kvh, c * P:(c + 1) * P],
                                         q_rhs, start=True, stop=True)
                        sc = ptmp.tile([P, T2], bf16, name=f"sc{l}_{g}_{c}",
                                       tag="sc")
                        nc.vector.tensor_tensor(sc[:], ps_s[:],
                                                maskc_sb[:, c, :], OP.add)
                        nc.scalar.activation(ex2d[:, c, :], sc[:], AF.Exp,
                                             bias=biasj_sb[:, c:c + 1])
                    exd[g] = ex2d
                for g in range(H // 2):
                    kvh = g // 2
                    q_rhs = q64[:, 2 * g:2 * g + 2, :]           # [64, 2, T]
                    ps_o = pp_o.tile([65, T2], f32, name=f"po{l}_{g}", tag="pso")
                    pend = None   # (v_pair, ex2, is_first)
                    for p_ in range(5):
                        if p_ == 0:
                            ex2 = exd[g]
                            v_pair = vtm8[:, 0:2, 65 * kvh:65 * kvh + 65]
                        else:
                            b2 = p_ - 1
                            ex2 = pexp.tile([P, 2, T2], f8,
                                            name=f"ex{l}_{g}_{b2}", tag="exp")
                            for half in range(2):
                                j = 2 * b2 + half
                                ps_s = pp_mm.tile([P, T2], f32,
                                                  name=f"psc{l}_{g}_{j}", tag="mmt")
                                nc.tensor.matmul(
                                    ps_s[:], kg[:, b2, kvh, half * P:(half + 1) * P],
                                    q_rhs, start=True, stop=True)
                                nc.scalar.activation(ex2[:, half, :], ps_s[:],
                                                     AF.Exp,
                                                     bias=biasj_sb[:, 2 + j:3 + j])
                            v_pair = vg8[:, b2, 0:2, 65 * kvh:65 * kvh + 65]
                        if pend is not None:
                            nc.tensor.matmul(ps_o[:], pend[0], pend[1][:],
                                             start=pend[2], stop=False,
                                             perf_mode=DR)
                        pend = (v_pair, ex2, p_ == 0)
                    nc.tensor.matmul(ps_o[:], pend[0], pend[1][:],
                                     start=False, stop=True, perf_mode=DR)
                    # per-pair sums row -> sums[2g:2g+2, :]; o -> o_sb
                    stg = ptmp.tile([1, T2], f32, name=f"stg{l}_{g}",
                                    tag="rowtmp")
                    nc.vector.tensor_copy(stg[:], ps_o[64:65, :])
                    nc.sync.dma_start(sums[2 * g:2 * g + 2, :], stg[:])
                    nc.vector.tensor_copy(o_sb[0:64, g, :], ps_o[0:64, 0:T])
                    nc.vector.tensor_copy(o_sb[64:P, g, :], ps_o[0:64, T:T2])
                sums_bf = pact.tile([12, T], bf16, name=f"sumsbf{l}", tag="sumsbf")
                with nc.allow_low_precision(reason="bf16 attn normalization scale"):
                    nc.vector.reciprocal(sums_bf[:], sums[:])

                # obf = AS * attn_out in fp8 (the AS is folded into sel: 16/sum)
                obf = pact.tile([P, DT, T], f8, name=f"obf{l}", tag="obf")
                for i in range(DT):
                    ps_b = pp_mm.tile([P, T], f32, name=f"pb{l}_{i}", tag="mmt")
                    nc.tensor.matmul(ps_b[:], sel_sb[:, i * P:(i + 1) * P], sums_bf[:],
                                     start=True, stop=True)
                    nc.vector.tensor_tensor(obf[:, i, :], o_sb[:, i, :], ps_b[:],
                                            OP.mult)

                # ---- o-projection + residual ----
                for m in range(DT):
                    ps = pp_mm.tile([P, T], f32, name=f"op{l}_{m}", tag="mmt")
                    mm8(ps[:], wo_sb, m * P, P, obf)
                    ot = ptmp.tile([P, T], bf16, name=f"ot{l}_{m}", tag="otmp")
                    nc.scalar.activation(ot[:], ps[:], AF.Identity,
                                         bias=bo_sb[:, m:m + 1], scale=D1)
                    nc.vector.tensor_tensor(x_sb[:, m, :], x_sb[:, m, :], ot[:],
                                            OP.add)

                # ---- FFN (pairs of 128-col tiles; sigmoid on [128, 2T]) ----
                h2 = pact.tile([P, DT, T], f8, name=f"h2_{l}", tag="h1")
                norm_cast(x_sb, h2, f"n2l{l}")

                ffa = pact.tile([P, HT, T], f8, name=f"ffa{l}", tag="ffa")
                HH = HT // 2
                for hf in range(2):
                    w1_sb, vw_sb = w1h[hf], vwh[hf]
                    for tp in range(HH // 2):        # pairs of col tiles
                        t0 = hf * HH + 2 * tp
                        ps_g = pp_mm.tile([P, 2 * T], f32, name=f"pg{l}_{t0}",
                                          tag="mmt")
                        ps_v = pp_mm.tile([P, 2 * T], f32, name=f"pv{l}_{t0}",
                                          tag="mmt")
                        for half in range(2):
                            cl = (2 * tp + half) * P
                            mm8(ps_g[:, half * T:(half + 1) * T], w1_sb, cl, P, h2)
                            mm8(ps_v[:, half * T:(half + 1) * T], vw_sb, cl, P, h2)
                        sil = ptmp.tile([P, 2 * T], f32, name=f"sil{l}_{t0}",
                                        tag="sil")
                        nc.scalar.activation(sil[:], ps_g[:], AF.Silu,
                                             scale=D1)
                        nc.vector.scalar_tensor_tensor(
                            ffa[:, t0:t0 + 2, :].rearrange("p a t -> p (a t)"),
                            ps_v[:], FS * D1, sil[:], OP.mult, OP.mult)

                for m in range(DT):
                    ps = pp_mm.tile([P, T], f32, name=f"p2{l}_{m}", tag="mmt")
                    for hf in range(2):
                        for kk in range(HH // 2):
                            nc.tensor.matmul(
                                ps[:], w2h[hf][:, 2 * kk:2 * kk + 2, m * P:(m + 1) * P],
                                ffa[:, hf * HH + 2 * kk:hf * HH + 2 * kk + 2, :],
                                start=(hf == 0 and kk == 0),
                                stop=(hf == 1 and kk == HH // 2 - 1),
                                perf_mode=DR)
                    nc.vector.scalar_tensor_tensor(
                        x_sb[:, m, :], ps[:], 1.0 / (FS * WS), x_sb[:, m, :],
                        OP.mult, OP.add)

            # ---------------- LM head: local tokens x full vocab ----------------
            xn = pact.tile([P, DT, T], f8, name="xn", tag="h1")
            norm_cast(x_sb, xn, "fin")

            # label logits: sum_f xn[f, t] * wlab[f, t]   (wlab carries 1/AS)
            ps_l = pp_o.tile([1, T], f32, name="psl", tag="pso")
            for kk in range(DT):
                tl = ptmp.tile([P, T], bf16, name=f"tl{kk}", tag="tl")
                nc.vector.tensor_tensor(tl[:], xn[:, kk, :], wlab_sb[:, kk, :],
                                        OP.mult)
                nc.tensor.matmul(ps_l[:], ones_bf[:], tl[:],
                                 start=(kk == 0), stop=(kk == DT - 1))
            lab_sb = pstate.tile([1, T], f32, name="lab_sb", tag="labsb")
            nc.vector.tensor_copy(lab_sb[:], ps_l[:])

            # full-vocab sum(exp(logits)) over local tokens; the ones-reduce
            # of tile vt is emitted after the logit chain of vt+1 so PE never
            # waits on the Exp.
            ps_S = pp_o.tile([1, T], f32, name="psS", tag="pso")
            pend_et = None
            for vt in range(VT):
                wt = pw.tile([P, DT, P], f8, name=f"lmw_{vt}", tag="lmwt",
                             bufs=6)
                nc.sync.dma_start(wt[:], lmw_d[vt])
                ps_lg = pp_mm.tile([P, T], f32, name=f"plg{vt}", tag="mmt")
                mm8(ps_lg[:], wt, 0, P, xn)
                et = pexp.tile([P, T], bf16, name=f"et{vt}", tag="exp")
                nc.scalar.activation(et[:], ps_lg[:], AF.Exp,
                                     bias=lmbv_sb[:, vt:vt + 1], scale=D1)
                if pend_et is not None:
                    nc.tensor.matmul(ps_S[:], ones_bf[:], pend_et,
                                     start=(vt == 1), stop=False)
                pend_et = et[:]
            nc.tensor.matmul(ps_S[:], ones_bf[:], pend_et,
                             start=False, stop=True)

            lg = pstate.tile([1, T], f32, name="lg", tag="lgsb")
            nc.scalar.activation(lg[:], ps_S[:], AF.Ln)
            nc.vector.tensor_tensor(lg[:], lg[:], lab_sb[:], OP.subtract)
            nc.vector.tensor_tensor(lg[:], lg[:], lmblab_sb[:], OP.subtract)
            nc.sync.dma_start(nll_d[:], lg[:])

        if hw_loop is not None:
            with tc.For_i(0, hw_loop) as _i:
                model_body(0)
        else:
            for rep in range(reps):
                model_body(rep)

    nc.compile()
    return nc


# ---------------- host-side sharding / input prep ----------------

def _feature_major(a2d):
    """[N, T] -> [128, N/128, T] device layout"""
    n, t = a2d.shape
    return np.ascontiguousarray(a2d.reshape(n // P, P, t).transpose(1, 0, 2))


def _f8(a):
    return np.clip(a, -240.0, 240.0).astype(F8)


_LMW_CACHE = {}


def prepare_inputs(inputs, num_layers=L):
    inp = {k: np.asarray(v) for k, v in inputs.items()}
    for k in ("wq", "bq", "wk", "bk", "wv", "bv", "wo", "bo",
              "n1", "n2", "w1", "vw", "w2"):
        inp[k] = inp[k][:num_layers]
    emb, lmw, lmb = inp["emb"], inp["lmw"], inp["lmb"]
    tgt, am, labels = inp["tgt"], inp["attention_mask"], inp["labels"]

    # rope pair deinterleave (evens then odds within each head), plus q-head
    # reorder so the 4 heads sharing each kv head sit in consecutive slots
    # (head h uses kv head h % 3; slots 4k..4k+3 hold heads {k, k+3, k+6, k+9}).
    NH = [0, 3, 6, 9, 1, 4, 7, 10, 2, 5, 8, 11]
    perm64 = np.concatenate([np.arange(0, HD, 2), np.arange(1, HD, 2)])
    qperm = np.concatenate([64 * NH[s] + perm64 for s in range(H)])
    operm = np.concatenate([64 * NH[s] + np.arange(HD) for s in range(H)])
    kperm = np.concatenate([64 * h + perm64 for h in range(KV)])

    def wdev(w, ko):
        nl, nin, nout = w.shape
        return np.ascontiguousarray(
            w.reshape(nl, ko, P, nout).transpose(0, 2, 1, 3))

    # norm weights fold into the input rows of the following projection;
    # fp8 weights carry scale WS
    n1w = inp["n1"][:, :, None].astype(np.float32)
    n2w = inp["n2"][:, :, None].astype(np.float32)
    wq = _f8(wdev((inp["wq"] * n1w)[:, :, qperm], DT) * WS)
    wk = _f8(wdev((inp["wk"] * n1w)[:, :, kperm], DT) * WS)
    wv = _f8(wdev(inp["wv"] * n1w, DT) * WS)
    wo = _f8(wdev(inp["wo"][:, operm, :], DT) * WS)
    w1 = _f8(wdev(inp["w1"] * n2w, DT) * WS)
    vw = _f8(wdev(inp["vw"] * n2w, DT) * WS)
    w2 = _f8(wdev(inp["w2"], HT) * WS)

    bq = np.ascontiguousarray(
        inp["bq"][:, qperm].reshape(num_layers, DT, P).transpose(0, 2, 1)).astype(np.float32)
    bo = np.ascontiguousarray(
        inp["bo"].reshape(num_layers, DT, P).transpose(0, 2, 1)).astype(np.float32)
    bk = np.zeros((num_layers, P, 2), np.float32)
    bkp = inp["bk"][:, kperm]
    bk[:, :, 0] = bkp[:, :P]
    bk[:, :64, 1] = bkp[:, P:]
    bv = np.zeros((num_layers, P, 2), np.float32)
    bv[:, :, 0] = inp["bv"][:, :P]
    bv[:, :64, 1] = inp["bv"][:, P:]
    normw = inp["normw"].astype(np.float32)

    thetas = np.power(10000.0, -2.0 * np.arange(0, HD, 2) / HD).astype(np.float32)
    sel = np.zeros((12, D), np.float32)
    for h in range(H):
        sel[h, 64 * h:64 * h + 64] = AS      # AS folds into the 1/sum bcast
    sel = sel.astype(BF)

    # full-vocab LM head (identical on every core); normw folds into rows
    key = (id(inputs.get("lmw")), num_layers)
    if key in _LMW_CACHE:
        lmw_dev, lmbv, lmwn = _LMW_CACHE[key]
    else:
        lmwn = lmw * normw[:, None]
        lmw_pad = np.zeros((D, VPAD), np.float32)
        lmw_pad[:, :V] = lmwn * WS
        lmw_fm = lmw_pad.reshape(DT, P, VPAD).transpose(1, 0, 2)   # [128, 6, VPAD]
        lmw_dev = _f8(np.ascontiguousarray(
            lmw_fm.reshape(P, DT, VT, P).transpose(2, 0, 1, 3)))   # [VT, P, DT, P]
        lmb_pad = np.full((VPAD,), NEG, np.float32)
        lmb_pad[:V] = lmb
        lmbv = np.ascontiguousarray(lmb_pad.reshape(VT, P).T).astype(np.float32)
        _LMW_CACHE.clear()
        _LMW_CACHE[key] = (lmw_dev, lmbv, lmwn)

    # shifted labels per batch row
    lab_full = np.concatenate([labels[:, 1:],
                               np.full((B, 1), EOS, labels.dtype)], axis=1)

    in_maps = []
    for c in range(NC):
        b, r = c // R, c % R
        pos = r * T + np.arange(T)

        tok = np.asarray(tgt[b, r * T:(r + 1) * T])
        x0 = _feature_major(emb[tok].T.astype(np.float32))

        ang = pos[None, :].astype(np.float32) * thetas[:, None]  # [32, T]
        cosv, sinv = np.cos(ang), np.sin(ang)
        C1 = np.tile(cosv, (4, 1)).astype(np.float32)
        C2 = np.concatenate([-sinv, sinv, -sinv, sinv], axis=0).astype(np.float32)

        # within-tile causal masks for the two diagonal tiles, duplicated for
        # the packed head pair -> [P, 2, 2T]
        pp_ = np.arange(P)[:, None]
        tt_ = np.arange(T)[None, :]
        maskc1 = np.stack([np.where(tt_ >= pp_, 0.0, NEG),
                           np.where(tt_ >= P + pp_, 0.0, NEG)],
                          axis=1).astype(np.float32)   # [P, 2, T]
        maskc = np.concatenate([maskc1, maskc1], axis=2)  # [P, 2, 2T]
        amk = np.asarray(am[b]) != 0
        biasj = np.full((P, 2 + 2 * R), NEG, np.float32)
        for c_ in range(2):
            keyi = P * (2 * r + c_) + np.arange(P)
            biasj[:, c_] = np.where(amk[keyi], 0.0, NEG)
        for j in range(2 * R):
            keyi = P * j + np.arange(P)
            alive = (j < 2 * r) & amk[keyi]
            biasj[:, 2 + j] = np.where(alive, 0.0, NEG)
        biasj -= np.log(64.0)   # fp8 exp scale: exp(s)/64, cancels in o/sums

        lab_b = np.asarray(lab_full[b, r * T:(r + 1) * T]).astype(np.int64)
        wlab = np.ascontiguousarray(
            (lmwn[:, lab_b] / AS).reshape(DT, P, T).transpose(1, 0, 2)).astype(BF)
        lmblab = lmb[lab_b].astype(np.float32)[None, :]

        in_maps.append({
            "x0": x0,
            "wq": wq, "wk": wk, "wv": wv, "wo": wo,
            "w1": w1, "vw": vw, "w2": w2,
            "bq": bq, "bk": bk, "bv": bv, "bo": bo,
            "c1q": (C1 / 8.0).astype(BF),
            "c2q": (C2 / 8.0).astype(BF),
            "c1k": C1.astype(BF), "c2k": C2.astype(BF),
            "maskc": maskc, "biasj": biasj, "sel": sel,
            "lmw": lmw_dev, "lmbv": lmbv,
            "wlab": wlab, "lmblab": lmblab,
        })
    return in_maps


_NC_CACHE = {}


def get_program(num_layers=L, reps=1):
    key = (num_layers, reps)
    if key not in _NC_CACHE:
        _NC_CACHE[key] = build_program(num_layers, reps=reps)
    return _NC_CACHE[key]


def kernel(**inputs) -> np.ndarray:
    nc = get_program(L)
    in_maps = prepare_inputs(inputs, L)
    res = bass_utils.run_bass_kernel_spmd(nc, in_maps, core_ids=list(range(NC)))
    tot = np.float64(0.0)
    for c in range(NC):
        tot += np.float64(res.results[c]["nll"].sum())
    return np.float32(tot / (B * S))
